# revision 14
# baseline (speedup 1.0000x reference)
"""DiceLoss Trainium2 kernel (sorted-pixel design).

Math: for preds [B,C,H,W] (logits), integer targets [B,H,W]:
  P = softmax over C;  S_c = sum_n P_nc;  D_c = sum_{n: t_n=c} P_{n,t_n}
  N_c = count(target==c); then TP/FP/FN -> alpha -> dice -> loss (host, 32-vec).

Device layout (per core = one batch): 128 SBUF partitions = 4 pixel
groups x 32 classes; free dim = pixels. The HOST SORTS pixels by target
class within each group, so "target == c" becomes a contiguous index
range per (group, class) row. Per tile [128, K]:
  ACT : E = exp(X) -> bf16
  PE  : Zrep = blockdiag(1s) @ E  (per-pixel sum over classes, PSUM)
  R   : 1/Z -> bf16, via custom-DVE reciprocal_approx (some tiles) or
        ACT Ln -> Exp(-L) (other tiles) to balance engine load
  DVE : DICE_RANGE_REDUCE(E, R, [0,K))      -> S partials [128,1]
  DVE : DICE_RANGE_REDUCE(E, R, [lo,hi))    -> D partials [128,1]
Host reduces the [128, n_tiles] partials and finishes the algebra.
"""

import numpy as np
import ml_dtypes

import concourse.bass as bass
import concourse.bacc as bacc
import concourse.mybir as mybir
from concourse.tile import TileContext
from concourse.bass_utils import run_bass_kernel_spmd

from dice_op import DICE_RANGE_REDUCE

# Problem shapes (hardcoded per contract).
B, C, H, W = 8, 32, 512, 512
HW = H * W            # 262144 pixels per batch
G = 4                 # pixel groups sharing the 128 partitions
GPIX = HW // G        # 65536 pixels per group
EPS = 1e-8
SMOOTH = 1e-5
NCORES = 8

F32 = mybir.dt.float32
BF16 = mybir.dt.bfloat16
BF = ml_dtypes.bfloat16


def build_nc(gpix=GPIX, k=2048, n_dve_recip=6):
    """One-core SPMD program. Every n-th tile computes 1/Z on the DVE
    (reciprocal_approx); the rest use the ACT Ln->Exp chain, balancing the
    two engines."""
    from concourse.dve_ops import RECIP_APPROX_FAST_CONSTS, RECIPROCAL_APPROX_FAST

    nt = gpix // k
    nc = bacc.Bacc("TRN2", target_bir_lowering=False)
    x = nc.declare_dram_parameter("x", [128, gpix], F32, isOutput=False)
    lo = nc.declare_dram_parameter("lo", [128, nt], F32, isOutput=False)
    hi = nc.declare_dram_parameter("hi", [128, nt], F32, isOutput=False)
    w1 = nc.declare_dram_parameter("w1", [128, 128], BF16, isOutput=False)
    s_out = nc.declare_dram_parameter("s_out", [128, nt], F32, isOutput=True)
    d_out = nc.declare_dram_parameter("d_out", [128, nt], F32, isOutput=True)

    cst = RECIP_APPROX_FAST_CONSTS
    MMF = 512  # columns per matmul instruction
    # spread the DVE-recip tiles evenly
    dve_tiles = set(round(i * (nt - 1) / max(n_dve_recip - 1, 1)) for i in range(n_dve_recip)) if n_dve_recip else set()

    with TileContext(nc) as tc:
        with (
            tc.tile_pool(name="const", bufs=1) as constp,
            tc.tile_pool(name="xin", bufs=3) as xp,
            tc.tile_pool(name="ework", bufs=3) as ep,
            tc.tile_pool(name="lwork", bufs=2) as lp,
            tc.tile_pool(name="rwork", bufs=2) as rp,
            tc.tile_pool(name="junk", bufs=2) as jp,
            tc.tile_pool(name="acc", bufs=1) as accp,
            tc.tile_pool(name="ps1", bufs=2, space="PSUM") as ps1,
        ):
            w1_t = constp.tile([128, 128], BF16)
            nc.sync.dma_start(out=w1_t[:], in_=w1[:])
            lo_t = constp.tile([128, nt], F32)
            nc.sync.dma_start(out=lo_t[:], in_=lo[:])
            hi_t = constp.tile([128, nt], F32)
            nc.sync.dma_start(out=hi_t[:], in_=hi[:])
            s_acc = accp.tile([128, nt], F32)
            d_acc = accp.tile([128, nt], F32)

            for t in range(nt):
                xt = xp.tile([128, k], F32)
                nc.sync.dma_start(out=xt[:], in_=x[:, t * k:(t + 1) * k])
                et = ep.tile([128, k], BF16)
                nc.scalar.activation(et[:], xt[:], mybir.ActivationFunctionType.Exp)
                z_ps = ps1.tile([128, k], F32)
                for m0 in range(0, k, MMF):
                    nc.tensor.matmul(
                        z_ps[:, m0:m0 + MMF], w1_t[:], et[:, m0:m0 + MMF],
                        start=True, stop=True,
                    )
                rt = rp.tile([128, k], BF16, tag="r")
                if t in dve_tiles:
                    nc.vector._custom_dve(
                        RECIPROCAL_APPROX_FAST, out=rt[:], in0=z_ps[:],
                        s0=cst["s0"], s1=cst["s1"], imm2=cst["imm2"],
                    )
                else:
                    lt = lp.tile([128, k], F32, tag="l")
                    nc.scalar.activation(lt[:], z_ps[:], mybir.ActivationFunctionType.Ln)
                    nc.scalar.activation(rt[:], lt[:], mybir.ActivationFunctionType.Exp,
                                         scale=-1.0)
                j1 = jp.tile([128, k], BF16, tag="j1")
                nc.vector._custom_dve(
                    DICE_RANGE_REDUCE, out=j1[:], in0=et[:], in1=rt[:],
                    s0=0.0, s1=float(k), accum_out=s_acc[:, t:t + 1],
                )
                j2 = jp.tile([128, k], BF16, tag="j2")
                nc.vector._custom_dve(
                    DICE_RANGE_REDUCE, out=j2[:], in0=et[:], in1=rt[:],
                    s0=lo_t[:, t:t + 1], s1=hi_t[:, t:t + 1],
                    accum_out=d_acc[:, t:t + 1],
                )
            nc.sync.dma_start(out=s_out[:], in_=s_acc[:])
            nc.sync.dma_start(out=d_out[:], in_=d_acc[:])
    nc.finalize()
    return nc


def host_w1():
    w1 = np.zeros((128, 128), dtype=BF)
    for g in range(G):
        w1[g * 32:(g + 1) * 32, g * 32:(g + 1) * 32] = BF(1.0)
    return w1


def host_prep(preds_b, targets_b, gpix=GPIX, k=2048):
    """Sort pixels by target within each group; build x [128, gpix] plus
    per-tile class-range bounds lo/hi [128, nt]."""
    nt = gpix // k
    xc = preds_b.reshape(C, G, gpix)
    tg = targets_b.reshape(G, gpix)
    x = np.empty((G * C, gpix), dtype=np.float32)
    lo = np.zeros((G * C, nt), dtype=np.float32)
    hi = np.zeros((G * C, nt), dtype=np.float32)
    for g in range(G):
        perm = np.argsort(tg[g], kind="stable")
        x[g * C:(g + 1) * C, :] = xc[:, g, :][:, perm]
        counts = np.bincount(tg[g].astype(np.int64), minlength=C)
        ends = np.cumsum(counts)
        starts = ends - counts
        for t in range(nt):
            lo[g * C:(g + 1) * C, t] = np.clip(starts - t * k, 0, k)
            hi[g * C:(g + 1) * C, t] = np.clip(ends - t * k, 0, k)
    return x, lo, hi


def finish_loss(S, D, Ncnt, npix_total):
    """Host-side 32-vector algebra, mirrors the reference exactly."""
    S = S.astype(np.float64)
    D = D.astype(np.float64)
    Ncnt = Ncnt.astype(np.float64)
    TP = EPS * S + (1.0 - EPS) * D
    FP = S - TP
    FN = (EPS * npix_total + (1.0 - EPS) * Ncnt) - TP
    alpha = np.clip(FP / (FP + FN + SMOOTH), 0.2, 0.8)
    beta = 1.0 - alpha
    den = TP + alpha * FP + beta * FN
    dice = TP / (den + SMOOTH)
    loss = np.sum(1.0 - dice) / C
    return np.float32(loss)


_NC_CACHE = {}


def _get_nc():
    if "nc" not in _NC_CACHE:
        _NC_CACHE["nc"] = build_nc()
    return _NC_CACHE["nc"]


def kernel(preds, targets):
    preds = np.asarray(preds, dtype=np.float32)
    targets = np.asarray(targets)
    nc = _get_nc()
    w1 = host_w1()
    in_maps = []
    for b in range(NCORES):
        x, lo, hi = host_prep(preds[b].reshape(C, HW), targets[b].reshape(-1))
        in_maps.append({"x": x, "lo": lo, "hi": hi, "w1": w1})
    res = run_bass_kernel_spmd(nc, in_maps, list(range(NCORES))).results
    S = np.zeros(C, dtype=np.float64)
    D = np.zeros(C, dtype=np.float64)
    for b in range(NCORES):
        so = np.asarray(res[b]["s_out"], dtype=np.float64)  # [128, nt]
        do = np.asarray(res[b]["d_out"], dtype=np.float64)
        S += so.sum(axis=1).reshape(G, C).sum(axis=0)
        D += do.sum(axis=1).reshape(G, C).sum(axis=0)
    Ncnt = np.bincount(targets.reshape(-1).astype(np.int64), minlength=C).astype(np.float64)
    return finish_loss(S, D, Ncnt, preds.shape[0] * HW)


# revision 20
# speedup vs baseline: 432.5890x; 432.5890x over previous
"""DiceLoss Trainium2 kernel (sorted-pixel design).

Math: for preds [B,C,H,W] (logits), integer targets [B,H,W]:
  P = softmax over C;  S_c = sum_n P_nc;  D_c = sum_{n: t_n=c} P_{n,t_n}
  N_c = count(target==c); then TP/FP/FN -> alpha -> dice -> loss (host, 32-vec).

Device layout (per core = one batch): 128 SBUF partitions = 4 pixel
groups x 32 classes; free dim = pixels. The HOST SORTS pixels by target
class within each group, so "target == c" becomes a contiguous index
range per (group, class) row. Per tile [128, K]:
  ACT : E = exp(X) -> bf16
  PE  : Zrep = blockdiag(1s) @ E  (per-pixel sum over classes, PSUM)
  R   : 1/Z -> bf16, via custom-DVE reciprocal_approx (some tiles) or
        ACT Ln -> Exp(-L) (other tiles) to balance engine load
  DVE : DICE_RANGE_REDUCE(E, R, [0,K))      -> S partials [128,1]
  DVE : DICE_RANGE_REDUCE(E, R, [lo,hi))    -> D partials [128,1]
Host reduces the [128, n_tiles] partials and finishes the algebra.
"""

import numpy as np
import ml_dtypes

import concourse.bass as bass
import concourse.bacc as bacc
import concourse.mybir as mybir
from concourse.tile import TileContext
from concourse.bass_utils import run_bass_kernel_spmd

# ---- custom DVE op: range-masked multiply-reduce (self-contained) ---------
# accum_out[p] = sum_k (in0[p,k]*in1[p,k]) * (s0[p] <= k < s1[p])


def _make_dice_range_op():
    import re
    import concourse.dve_ops as dve_ops
    from concourse.dve_ops import DveOp
    from concourse.dve_spec import Spec, Src0, Src1, C0, C1, Idx, Zero
    from operator import add

    name = "DICE_RANGE_REDUCE"
    if name in dve_ops._SUB_OPCODE_FOR_NAME:
        for op in dve_ops.OPS:
            if op.name == name:
                return op

    def _ref(in0, in1, s0, s1, imm2):
        n = in0.shape[-1]
        idx = np.arange(n, dtype=np.float32)
        s0 = np.asarray(s0, dtype=np.float32).reshape(-1, 1)
        s1 = np.asarray(s1, dtype=np.float32).reshape(-1, 1)
        mask = ((idx >= s0) & (idx < s1)).astype(np.float32)
        b = (in0.astype(np.float32) * in1.astype(np.float32) * mask).astype(np.float32)
        return b, b.reshape(b.shape[0], -1).sum(axis=-1, keepdims=True)

    spec = Spec(
        body=(Src0 * Src1) * ((Idx >= C0) & (Idx < C1)),
        accum=add,
        accum_init=Zero,
        reference=_ref,
    )
    row = dve_ops._CUSTOM_DVE_ROW_BASE + len(dve_ops.OPS)
    assert row < 0x20
    op = DveOp(name, spec, subdim=False, uops_sha={})
    dve_ops.OPS.append(op)
    dve_ops.CUSTOM_DVE_SPECS[name] = spec
    dve_ops._SUB_OPCODE_FOR_NAME[name] = row
    for ver in ("v3", "v4"):
        try:
            op.compile(ver)
        except ValueError as e:
            m = re.search(r'uops_sha\["%s"\]="([0-9a-f]+)"' % ver, str(e))
            if not m:
                raise
            op.uops_sha[ver] = m.group(1)
            dve_ops._COMPILE_CACHE.pop((name, ver), None)
        op.compile(ver)
    return op


DICE_RANGE_REDUCE = _make_dice_range_op()

# Problem shapes (hardcoded per contract).
B, C, H, W = 8, 32, 512, 512
HW = H * W            # 262144 pixels per batch
G = 4                 # pixel groups sharing the 128 partitions
GPIX = HW // G        # 65536 pixels per group
EPS = 1e-8
SMOOTH = 1e-5
NCORES = 8

F32 = mybir.dt.float32
BF16 = mybir.dt.bfloat16
BF = ml_dtypes.bfloat16


def _patch_act_tables():
    """Order activation tables so the set containing BOTH Exp and Ln is
    preferred - otherwise the table-load pass thrashes between the exp-only
    and ln-only sets (one ~2.7us reload per tile)."""
    import concourse.bacc as _bacc
    if getattr(_bacc, "_dice_tables_patched", False):
        return
    orig = _bacc.get_activation_tables

    def filtered(arch):
        # PRESERVE dict order (set ids are positional indexes into
        # act_info.json) - only hide Exp/Ln from the single-function sets so
        # the pass must pick the combined one.
        tabs = dict(orig(arch))
        if "natural_log_exp_and_others" not in tabs:
            return tabs
        import concourse.mybir as mb
        out = {}
        for name, funcs in tabs.items():
            if name != "natural_log_exp_and_others":
                funcs = {f for f in funcs
                         if f not in (mb.ActivationFunctionType.Exp,
                                      mb.ActivationFunctionType.Ln)}
            out[name] = funcs
        return out

    _bacc.get_activation_tables = filtered
    _bacc._dice_tables_patched = True


def build_nc(gpix=GPIX, k=2048, n_dve_recip=10, process_nt=None):
    """One-core SPMD program. Every n-th tile computes 1/Z on the DVE
    (reciprocal_approx); the rest use the ACT Ln->Exp chain, balancing the
    two engines. process_nt: only emit compute for the first N tiles
    (benchmarking aid - transfers stay identical)."""
    from concourse.dve_ops import RECIP_APPROX_FAST_CONSTS, RECIPROCAL_APPROX_FAST

    _patch_act_tables()
    nt = gpix // k
    nc = bacc.Bacc("TRN2", target_bir_lowering=False)
    x = nc.declare_dram_parameter("x", [128, gpix], F32, isOutput=False)
    lo = nc.declare_dram_parameter("lo", [128, nt], F32, isOutput=False)
    hi = nc.declare_dram_parameter("hi", [128, nt], F32, isOutput=False)
    w1 = nc.declare_dram_parameter("w1", [128, 128], BF16, isOutput=False)
    s_out = nc.declare_dram_parameter("s_out", [128, nt], F32, isOutput=True)
    d_out = nc.declare_dram_parameter("d_out", [128, nt], F32, isOutput=True)

    cst = RECIP_APPROX_FAST_CONSTS
    MMF = 512  # columns per matmul instruction
    # spread the DVE-recip tiles evenly
    dve_tiles = set(round(i * (nt - 1) / max(n_dve_recip - 1, 1)) for i in range(n_dve_recip)) if n_dve_recip else set()

    with TileContext(nc) as tc:
        with (
            tc.tile_pool(name="const", bufs=1) as constp,
            tc.tile_pool(name="xin", bufs=3) as xp,
            tc.tile_pool(name="ework", bufs=4) as ep,
            tc.tile_pool(name="lwork", bufs=3) as lp,
            tc.tile_pool(name="rwork", bufs=3) as rp,
            tc.tile_pool(name="junk", bufs=3) as jp,
            tc.tile_pool(name="acc", bufs=1) as accp,
            tc.tile_pool(name="ps1", bufs=2, space="PSUM") as ps1,
        ):
            w1_t = constp.tile([128, 128], BF16)
            nc.sync.dma_start(out=w1_t[:], in_=w1[:])
            lo_t = constp.tile([128, nt], F32)
            nc.sync.dma_start(out=lo_t[:], in_=lo[:])
            hi_t = constp.tile([128, nt], F32)
            nc.sync.dma_start(out=hi_t[:], in_=hi[:])
            s_acc = accp.tile([128, nt], F32)
            d_acc = accp.tile([128, nt], F32)

            for t in range(nt if process_nt is None else process_nt):
                xt = xp.tile([128, k], F32)
                nc.sync.dma_start(out=xt[:], in_=x[:, t * k:(t + 1) * k])
                et = ep.tile([128, k], BF16)
                nc.scalar.activation(et[:], xt[:], mybir.ActivationFunctionType.Exp)
                z_ps = ps1.tile([128, k], F32)
                for m0 in range(0, k, MMF):
                    nc.tensor.matmul(
                        z_ps[:, m0:m0 + MMF], w1_t[:], et[:, m0:m0 + MMF],
                        start=True, stop=True,
                    )
                rt = rp.tile([128, k], BF16, tag="r")
                if t in dve_tiles:
                    nc.vector._custom_dve(
                        RECIPROCAL_APPROX_FAST, out=rt[:], in0=z_ps[:],
                        s0=cst["s0"], s1=cst["s1"], imm2=cst["imm2"],
                    )
                else:
                    lt = lp.tile([128, k], F32, tag="l")
                    nc.scalar.activation(lt[:], z_ps[:], mybir.ActivationFunctionType.Ln)
                    nc.scalar.activation(rt[:], lt[:], mybir.ActivationFunctionType.Exp,
                                         scale=-1.0)
                j1 = jp.tile([128, k], BF16, tag="j1")
                nc.vector._custom_dve(
                    DICE_RANGE_REDUCE, out=j1[:], in0=et[:], in1=rt[:],
                    s0=0.0, s1=float(k), accum_out=s_acc[:, t:t + 1],
                )
                j2 = jp.tile([128, k], BF16, tag="j2")
                nc.vector._custom_dve(
                    DICE_RANGE_REDUCE, out=j2[:], in0=et[:], in1=rt[:],
                    s0=lo_t[:, t:t + 1], s1=hi_t[:, t:t + 1],
                    accum_out=d_acc[:, t:t + 1],
                )
            nc.sync.dma_start(out=s_out[:], in_=s_acc[:])
            nc.sync.dma_start(out=d_out[:], in_=d_acc[:])
    nc.finalize()
    return nc


def host_w1():
    w1 = np.zeros((128, 128), dtype=BF)
    for g in range(G):
        w1[g * 32:(g + 1) * 32, g * 32:(g + 1) * 32] = BF(1.0)
    return w1


def host_prep(preds_b, targets_b, gpix=GPIX, k=2048):
    """Sort pixels by target within each group; build x [128, gpix] plus
    per-tile class-range bounds lo/hi [128, nt]."""
    nt = gpix // k
    xc = preds_b.reshape(C, G, gpix)
    tg = targets_b.reshape(G, gpix)
    x = np.empty((G * C, gpix), dtype=np.float32)
    lo = np.zeros((G * C, nt), dtype=np.float32)
    hi = np.zeros((G * C, nt), dtype=np.float32)
    for g in range(G):
        perm = np.argsort(tg[g], kind="stable")
        x[g * C:(g + 1) * C, :] = xc[:, g, :][:, perm]
        counts = np.bincount(tg[g].astype(np.int64), minlength=C)
        ends = np.cumsum(counts)
        starts = ends - counts
        for t in range(nt):
            lo[g * C:(g + 1) * C, t] = np.clip(starts - t * k, 0, k)
            hi[g * C:(g + 1) * C, t] = np.clip(ends - t * k, 0, k)
    return x, lo, hi


def finish_loss(S, D, Ncnt, npix_total):
    """Host-side 32-vector algebra, mirrors the reference exactly."""
    S = S.astype(np.float64)
    D = D.astype(np.float64)
    Ncnt = Ncnt.astype(np.float64)
    TP = EPS * S + (1.0 - EPS) * D
    FP = S - TP
    FN = (EPS * npix_total + (1.0 - EPS) * Ncnt) - TP
    alpha = np.clip(FP / (FP + FN + SMOOTH), 0.2, 0.8)
    beta = 1.0 - alpha
    den = TP + alpha * FP + beta * FN
    dice = TP / (den + SMOOTH)
    loss = np.sum(1.0 - dice) / C
    return np.float32(loss)


_NC_CACHE = {}


def _get_nc():
    if "nc" not in _NC_CACHE:
        _NC_CACHE["nc"] = build_nc()
    return _NC_CACHE["nc"]


def kernel(preds, targets):
    preds = np.asarray(preds, dtype=np.float32)
    targets = np.asarray(targets)
    nc = _get_nc()
    w1 = host_w1()
    in_maps = []
    for b in range(NCORES):
        x, lo, hi = host_prep(preds[b].reshape(C, HW), targets[b].reshape(-1))
        in_maps.append({"x": x, "lo": lo, "hi": hi, "w1": w1})
    res = run_bass_kernel_spmd(nc, in_maps, list(range(NCORES))).results
    S = np.zeros(C, dtype=np.float64)
    D = np.zeros(C, dtype=np.float64)
    for b in range(NCORES):
        so = np.asarray(res[b]["s_out"], dtype=np.float64)  # [128, nt]
        do = np.asarray(res[b]["d_out"], dtype=np.float64)
        S += so.sum(axis=1).reshape(G, C).sum(axis=0)
        D += do.sum(axis=1).reshape(G, C).sum(axis=0)
    Ncnt = np.bincount(targets.reshape(-1).astype(np.int64), minlength=C).astype(np.float64)
    return np.array(finish_loss(S, D, Ncnt, preds.shape[0] * HW), dtype=np.float32)


# revision 21
# speedup vs baseline: 438.0894x; 1.0127x over previous
"""DiceLoss Trainium2 kernel (sorted-pixel design).

Math: for preds [B,C,H,W] (logits), integer targets [B,H,W]:
  P = softmax over C;  S_c = sum_n P_nc;  D_c = sum_{n: t_n=c} P_{n,t_n}
  N_c = count(target==c); then TP/FP/FN -> alpha -> dice -> loss (host, 32-vec).

Device layout (per core = one batch): 128 SBUF partitions = 4 pixel
groups x 32 classes; free dim = pixels. The HOST SORTS pixels by target
class within each group, so "target == c" becomes a contiguous index
range per (group, class) row. Per tile [128, K]:
  ACT : E = exp(X) -> bf16
  PE  : Zrep = blockdiag(1s) @ E  (per-pixel sum over classes, PSUM)
  R   : 1/Z -> bf16, via custom-DVE reciprocal_approx (some tiles) or
        ACT Ln -> Exp(-L) (other tiles) to balance engine load
  DVE : DICE_RANGE_REDUCE(E, R, [0,K))      -> S partials [128,1]
  DVE : DICE_RANGE_REDUCE(E, R, [lo,hi))    -> D partials [128,1]
Host reduces the [128, n_tiles] partials and finishes the algebra.
"""

import numpy as np
import ml_dtypes

import concourse.bass as bass
import concourse.bacc as bacc
import concourse.mybir as mybir
from concourse.tile import TileContext
from concourse.bass_utils import run_bass_kernel_spmd

# ---- custom DVE op: range-masked multiply-reduce (self-contained) ---------
# accum_out[p] = sum_k (in0[p,k]*in1[p,k]) * (s0[p] <= k < s1[p])


def _make_dice_range_op():
    import re
    import concourse.dve_ops as dve_ops
    from concourse.dve_ops import DveOp
    from concourse.dve_spec import Spec, Src0, Src1, C0, C1, Idx, Zero
    from operator import add

    name = "DICE_RANGE_REDUCE"
    if name in dve_ops._SUB_OPCODE_FOR_NAME:
        for op in dve_ops.OPS:
            if op.name == name:
                return op

    def _ref(in0, in1, s0, s1, imm2):
        n = in0.shape[-1]
        idx = np.arange(n, dtype=np.float32)
        s0 = np.asarray(s0, dtype=np.float32).reshape(-1, 1)
        s1 = np.asarray(s1, dtype=np.float32).reshape(-1, 1)
        mask = ((idx >= s0) & (idx < s1)).astype(np.float32)
        b = (in0.astype(np.float32) * in1.astype(np.float32) * mask).astype(np.float32)
        return b, b.reshape(b.shape[0], -1).sum(axis=-1, keepdims=True)

    spec = Spec(
        body=(Src0 * Src1) * ((Idx >= C0) & (Idx < C1)),
        accum=add,
        accum_init=Zero,
        reference=_ref,
    )
    row = dve_ops._CUSTOM_DVE_ROW_BASE + len(dve_ops.OPS)
    assert row < 0x20
    op = DveOp(name, spec, subdim=False, uops_sha={})
    dve_ops.OPS.append(op)
    dve_ops.CUSTOM_DVE_SPECS[name] = spec
    dve_ops._SUB_OPCODE_FOR_NAME[name] = row
    for ver in ("v3", "v4"):
        try:
            op.compile(ver)
        except ValueError as e:
            m = re.search(r'uops_sha\["%s"\]="([0-9a-f]+)"' % ver, str(e))
            if not m:
                raise
            op.uops_sha[ver] = m.group(1)
            dve_ops._COMPILE_CACHE.pop((name, ver), None)
        op.compile(ver)
    return op


DICE_RANGE_REDUCE = _make_dice_range_op()

# Problem shapes (hardcoded per contract).
B, C, H, W = 8, 32, 512, 512
HW = H * W            # 262144 pixels per batch
G = 4                 # pixel groups sharing the 128 partitions
GPIX = HW // G        # 65536 pixels per group
EPS = 1e-8
SMOOTH = 1e-5
NCORES = 8

F32 = mybir.dt.float32
BF16 = mybir.dt.bfloat16
BF = ml_dtypes.bfloat16


def _patch_act_tables():
    """Order activation tables so the set containing BOTH Exp and Ln is
    preferred - otherwise the table-load pass thrashes between the exp-only
    and ln-only sets (one ~2.7us reload per tile)."""
    import concourse.bacc as _bacc
    if getattr(_bacc, "_dice_tables_patched", False):
        return
    orig = _bacc.get_activation_tables

    def filtered(arch):
        # PRESERVE dict order (set ids are positional indexes into
        # act_info.json) - only hide Exp/Ln from the single-function sets so
        # the pass must pick the combined one.
        tabs = dict(orig(arch))
        if "natural_log_exp_and_others" not in tabs:
            return tabs
        import concourse.mybir as mb
        out = {}
        for name, funcs in tabs.items():
            if name != "natural_log_exp_and_others":
                funcs = {f for f in funcs
                         if f not in (mb.ActivationFunctionType.Exp,
                                      mb.ActivationFunctionType.Ln)}
            out[name] = funcs
        return out

    _bacc.get_activation_tables = filtered
    _bacc._dice_tables_patched = True


def build_nc(gpix=GPIX, k=2048, n_dve_recip=10, process_nt=None):
    """One-core SPMD program. Every n-th tile computes 1/Z on the DVE
    (reciprocal_approx); the rest use the ACT Ln->Exp chain, balancing the
    two engines. process_nt: only emit compute for the first N tiles
    (benchmarking aid - transfers stay identical)."""
    from concourse.dve_ops import RECIP_APPROX_FAST_CONSTS, RECIPROCAL_APPROX_FAST

    _patch_act_tables()
    nt = gpix // k
    nc = bacc.Bacc("TRN2", target_bir_lowering=False)
    x = nc.declare_dram_parameter("x", [128, gpix], F32, isOutput=False)
    lo = nc.declare_dram_parameter("lo", [128, nt], F32, isOutput=False)
    hi = nc.declare_dram_parameter("hi", [128, nt], F32, isOutput=False)
    w1 = nc.declare_dram_parameter("w1", [128, 128], BF16, isOutput=False)
    s_out = nc.declare_dram_parameter("s_out", [128, nt], F32, isOutput=True)
    d_out = nc.declare_dram_parameter("d_out", [128, nt], F32, isOutput=True)

    cst = RECIP_APPROX_FAST_CONSTS
    MMF = 512  # columns per matmul instruction
    # spread the DVE-recip tiles evenly
    dve_tiles = set(round(i * (nt - 1) / max(n_dve_recip - 1, 1)) for i in range(n_dve_recip)) if n_dve_recip else set()

    with TileContext(nc) as tc:
        with (
            tc.tile_pool(name="const", bufs=1) as constp,
            tc.tile_pool(name="xin", bufs=4) as xp,
            tc.tile_pool(name="ework", bufs=6) as ep,
            tc.tile_pool(name="lwork", bufs=3) as lp,
            tc.tile_pool(name="rwork", bufs=3) as rp,
            tc.tile_pool(name="junk", bufs=3) as jp,
            tc.tile_pool(name="acc", bufs=1) as accp,
            tc.tile_pool(name="ps1", bufs=2, space="PSUM") as ps1,
        ):
            w1_t = constp.tile([128, 128], BF16)
            nc.sync.dma_start(out=w1_t[:], in_=w1[:])
            lo_t = constp.tile([128, nt], F32)
            nc.sync.dma_start(out=lo_t[:], in_=lo[:])
            hi_t = constp.tile([128, nt], F32)
            nc.sync.dma_start(out=hi_t[:], in_=hi[:])
            s_acc = accp.tile([128, nt], F32)
            d_acc = accp.tile([128, nt], F32)

            for t in range(nt if process_nt is None else process_nt):
                xt = xp.tile([128, k], F32)
                nc.sync.dma_start(out=xt[:], in_=x[:, t * k:(t + 1) * k])
                et = ep.tile([128, k], BF16)
                nc.scalar.activation(et[:], xt[:], mybir.ActivationFunctionType.Exp)
                z_ps = ps1.tile([128, k], F32)
                for m0 in range(0, k, MMF):
                    nc.tensor.matmul(
                        z_ps[:, m0:m0 + MMF], w1_t[:], et[:, m0:m0 + MMF],
                        start=True, stop=True,
                    )
                rt = rp.tile([128, k], BF16, tag="r")
                if t in dve_tiles:
                    nc.vector._custom_dve(
                        RECIPROCAL_APPROX_FAST, out=rt[:], in0=z_ps[:],
                        s0=cst["s0"], s1=cst["s1"], imm2=cst["imm2"],
                    )
                else:
                    lt = lp.tile([128, k], F32, tag="l")
                    nc.scalar.activation(lt[:], z_ps[:], mybir.ActivationFunctionType.Ln)
                    nc.scalar.activation(rt[:], lt[:], mybir.ActivationFunctionType.Exp,
                                         scale=-1.0)
                j1 = jp.tile([128, k], BF16, tag="j1")
                nc.vector._custom_dve(
                    DICE_RANGE_REDUCE, out=j1[:], in0=et[:], in1=rt[:],
                    s0=0.0, s1=float(k), accum_out=s_acc[:, t:t + 1],
                )
                j2 = jp.tile([128, k], BF16, tag="j2")
                nc.vector._custom_dve(
                    DICE_RANGE_REDUCE, out=j2[:], in0=et[:], in1=rt[:],
                    s0=lo_t[:, t:t + 1], s1=hi_t[:, t:t + 1],
                    accum_out=d_acc[:, t:t + 1],
                )
            nc.sync.dma_start(out=s_out[:], in_=s_acc[:])
            nc.sync.dma_start(out=d_out[:], in_=d_acc[:])
    nc.finalize()
    return nc


def host_w1():
    w1 = np.zeros((128, 128), dtype=BF)
    for g in range(G):
        w1[g * 32:(g + 1) * 32, g * 32:(g + 1) * 32] = BF(1.0)
    return w1


def host_prep(preds_b, targets_b, gpix=GPIX, k=2048):
    """Sort pixels by target within each group; build x [128, gpix] plus
    per-tile class-range bounds lo/hi [128, nt]."""
    nt = gpix // k
    xc = preds_b.reshape(C, G, gpix)
    tg = targets_b.reshape(G, gpix)
    x = np.empty((G * C, gpix), dtype=np.float32)
    lo = np.zeros((G * C, nt), dtype=np.float32)
    hi = np.zeros((G * C, nt), dtype=np.float32)
    for g in range(G):
        perm = np.argsort(tg[g], kind="stable")
        x[g * C:(g + 1) * C, :] = xc[:, g, :][:, perm]
        counts = np.bincount(tg[g].astype(np.int64), minlength=C)
        ends = np.cumsum(counts)
        starts = ends - counts
        for t in range(nt):
            lo[g * C:(g + 1) * C, t] = np.clip(starts - t * k, 0, k)
            hi[g * C:(g + 1) * C, t] = np.clip(ends - t * k, 0, k)
    return x, lo, hi


def finish_loss(S, D, Ncnt, npix_total):
    """Host-side 32-vector algebra, mirrors the reference exactly."""
    S = S.astype(np.float64)
    D = D.astype(np.float64)
    Ncnt = Ncnt.astype(np.float64)
    TP = EPS * S + (1.0 - EPS) * D
    FP = S - TP
    FN = (EPS * npix_total + (1.0 - EPS) * Ncnt) - TP
    alpha = np.clip(FP / (FP + FN + SMOOTH), 0.2, 0.8)
    beta = 1.0 - alpha
    den = TP + alpha * FP + beta * FN
    dice = TP / (den + SMOOTH)
    loss = np.sum(1.0 - dice) / C
    return np.float32(loss)


_NC_CACHE = {}


def _get_nc():
    if "nc" not in _NC_CACHE:
        _NC_CACHE["nc"] = build_nc()
    return _NC_CACHE["nc"]


def kernel(preds, targets):
    preds = np.asarray(preds, dtype=np.float32)
    targets = np.asarray(targets)
    nc = _get_nc()
    w1 = host_w1()
    in_maps = []
    for b in range(NCORES):
        x, lo, hi = host_prep(preds[b].reshape(C, HW), targets[b].reshape(-1))
        in_maps.append({"x": x, "lo": lo, "hi": hi, "w1": w1})
    res = run_bass_kernel_spmd(nc, in_maps, list(range(NCORES))).results
    S = np.zeros(C, dtype=np.float64)
    D = np.zeros(C, dtype=np.float64)
    for b in range(NCORES):
        so = np.asarray(res[b]["s_out"], dtype=np.float64)  # [128, nt]
        do = np.asarray(res[b]["d_out"], dtype=np.float64)
        S += so.sum(axis=1).reshape(G, C).sum(axis=0)
        D += do.sum(axis=1).reshape(G, C).sum(axis=0)
    Ncnt = np.bincount(targets.reshape(-1).astype(np.int64), minlength=C).astype(np.float64)
    return np.array(finish_loss(S, D, Ncnt, preds.shape[0] * HW), dtype=np.float32)


# revision 23
# speedup vs baseline: 442.8542x; 1.0109x over previous
"""DiceLoss Trainium2 kernel (sorted-pixel design).

Math: for preds [B,C,H,W] (logits), integer targets [B,H,W]:
  P = softmax over C;  S_c = sum_n P_nc;  D_c = sum_{n: t_n=c} P_{n,t_n}
  N_c = count(target==c); then TP/FP/FN -> alpha -> dice -> loss (host, 32-vec).

Device layout (per core = one batch): 128 SBUF partitions = 4 pixel
groups x 32 classes; free dim = pixels. The HOST SORTS pixels by target
class within each group, so "target == c" becomes a contiguous index
range per (group, class) row. Per tile [128, K]:
  ACT : E = exp(X) -> bf16
  PE  : Zrep = blockdiag(1s) @ E  (per-pixel sum over classes, PSUM)
  R   : 1/Z -> bf16, via custom-DVE reciprocal_approx (some tiles) or
        ACT Ln -> Exp(-L) (other tiles) to balance engine load
  DVE : DICE_RANGE_REDUCE(E, R, [0,K))      -> S partials [128,1]
  DVE : DICE_RANGE_REDUCE(E, R, [lo,hi))    -> D partials [128,1]
Host reduces the [128, n_tiles] partials and finishes the algebra.
"""

import numpy as np
import ml_dtypes

import concourse.bass as bass
import concourse.bacc as bacc
import concourse.mybir as mybir
from concourse.tile import TileContext
from concourse.bass_utils import run_bass_kernel_spmd

# ---- custom DVE op: range-masked multiply-reduce (self-contained) ---------
# accum_out[p] = sum_k (in0[p,k]*in1[p,k]) * (s0[p] <= k < s1[p])


def _make_dice_range_op():
    import re
    import concourse.dve_ops as dve_ops
    from concourse.dve_ops import DveOp
    from concourse.dve_spec import Spec, Src0, Src1, C0, C1, Idx, Zero
    from operator import add

    name = "DICE_RANGE_REDUCE"
    if name in dve_ops._SUB_OPCODE_FOR_NAME:
        for op in dve_ops.OPS:
            if op.name == name:
                return op

    def _ref(in0, in1, s0, s1, imm2):
        n = in0.shape[-1]
        idx = np.arange(n, dtype=np.float32)
        s0 = np.asarray(s0, dtype=np.float32).reshape(-1, 1)
        s1 = np.asarray(s1, dtype=np.float32).reshape(-1, 1)
        mask = ((idx >= s0) & (idx < s1)).astype(np.float32)
        b = (in0.astype(np.float32) * in1.astype(np.float32) * mask).astype(np.float32)
        return b, b.reshape(b.shape[0], -1).sum(axis=-1, keepdims=True)

    spec = Spec(
        body=(Src0 * Src1) * ((Idx >= C0) & (Idx < C1)),
        accum=add,
        accum_init=Zero,
        reference=_ref,
    )
    row = dve_ops._CUSTOM_DVE_ROW_BASE + len(dve_ops.OPS)
    assert row < 0x20
    op = DveOp(name, spec, subdim=False, uops_sha={})
    dve_ops.OPS.append(op)
    dve_ops.CUSTOM_DVE_SPECS[name] = spec
    dve_ops._SUB_OPCODE_FOR_NAME[name] = row
    for ver in ("v3", "v4"):
        try:
            op.compile(ver)
        except ValueError as e:
            m = re.search(r'uops_sha\["%s"\]="([0-9a-f]+)"' % ver, str(e))
            if not m:
                raise
            op.uops_sha[ver] = m.group(1)
            dve_ops._COMPILE_CACHE.pop((name, ver), None)
        op.compile(ver)
    return op


DICE_RANGE_REDUCE = _make_dice_range_op()

# Problem shapes (hardcoded per contract).
B, C, H, W = 8, 32, 512, 512
HW = H * W            # 262144 pixels per batch
G = 4                 # pixel groups sharing the 128 partitions
GPIX = HW // G        # 65536 pixels per group
EPS = 1e-8
SMOOTH = 1e-5
NCORES = 8

F32 = mybir.dt.float32
BF16 = mybir.dt.bfloat16
BF = ml_dtypes.bfloat16


def _patch_act_tables():
    """Order activation tables so the set containing BOTH Exp and Ln is
    preferred - otherwise the table-load pass thrashes between the exp-only
    and ln-only sets (one ~2.7us reload per tile)."""
    import concourse.bacc as _bacc
    if getattr(_bacc, "_dice_tables_patched", False):
        return
    orig = _bacc.get_activation_tables

    def filtered(arch):
        # PRESERVE dict order (set ids are positional indexes into
        # act_info.json) - only hide Exp/Ln from the single-function sets so
        # the pass must pick the combined one.
        tabs = dict(orig(arch))
        if "natural_log_exp_and_others" not in tabs:
            return tabs
        import concourse.mybir as mb
        out = {}
        for name, funcs in tabs.items():
            if name != "natural_log_exp_and_others":
                funcs = {f for f in funcs
                         if f not in (mb.ActivationFunctionType.Exp,
                                      mb.ActivationFunctionType.Ln)}
            out[name] = funcs
        return out

    _bacc.get_activation_tables = filtered
    _bacc._dice_tables_patched = True


def build_nc(gpix=GPIX, k=2048, n_dve_recip=7, process_nt=None):
    """One-core SPMD program. Every n-th tile computes 1/Z on the DVE
    (reciprocal_approx); the rest use the ACT Ln->Exp chain, balancing the
    two engines. process_nt: only emit compute for the first N tiles
    (benchmarking aid - transfers stay identical)."""
    from concourse.dve_ops import RECIP_APPROX_FAST_CONSTS, RECIPROCAL_APPROX_FAST

    _patch_act_tables()
    nt = gpix // k
    nc = bacc.Bacc("TRN2", target_bir_lowering=False)
    x = nc.declare_dram_parameter("x", [128, gpix], F32, isOutput=False)
    lo = nc.declare_dram_parameter("lo", [128, nt], F32, isOutput=False)
    hi = nc.declare_dram_parameter("hi", [128, nt], F32, isOutput=False)
    w1 = nc.declare_dram_parameter("w1", [128, 128], BF16, isOutput=False)
    s_out = nc.declare_dram_parameter("s_out", [128, nt], F32, isOutput=True)
    d_out = nc.declare_dram_parameter("d_out", [128, nt], F32, isOutput=True)

    cst = RECIP_APPROX_FAST_CONSTS
    MMF = 512  # columns per matmul instruction
    # spread the DVE-recip tiles evenly
    dve_tiles = set(round(i * (nt - 1) / max(n_dve_recip - 1, 1)) for i in range(n_dve_recip)) if n_dve_recip else set()

    with TileContext(nc) as tc:
        with (
            tc.tile_pool(name="const", bufs=1) as constp,
            tc.tile_pool(name="xin", bufs=6) as xp,
            tc.tile_pool(name="ework", bufs=8) as ep,
            tc.tile_pool(name="lwork", bufs=3) as lp,
            tc.tile_pool(name="rwork", bufs=4) as rp,
            tc.tile_pool(name="junk", bufs=4) as jp,
            tc.tile_pool(name="acc", bufs=1) as accp,
            tc.tile_pool(name="ps1", bufs=2, space="PSUM") as ps1,
        ):
            w1_t = constp.tile([128, 128], BF16)
            nc.sync.dma_start(out=w1_t[:], in_=w1[:])
            lo_t = constp.tile([128, nt], F32)
            nc.sync.dma_start(out=lo_t[:], in_=lo[:])
            hi_t = constp.tile([128, nt], F32)
            nc.sync.dma_start(out=hi_t[:], in_=hi[:])
            s_acc = accp.tile([128, nt], F32)
            d_acc = accp.tile([128, nt], F32)

            for t in range(nt if process_nt is None else process_nt):
                xt = xp.tile([128, k], F32)
                nc.sync.dma_start(out=xt[:], in_=x[:, t * k:(t + 1) * k])
                et = ep.tile([128, k], BF16)
                nc.scalar.activation(et[:], xt[:], mybir.ActivationFunctionType.Exp)
                z_ps = ps1.tile([128, k], F32)
                for m0 in range(0, k, MMF):
                    nc.tensor.matmul(
                        z_ps[:, m0:m0 + MMF], w1_t[:], et[:, m0:m0 + MMF],
                        start=True, stop=True,
                    )
                rt = rp.tile([128, k], BF16, tag="r")
                if t in dve_tiles:
                    nc.vector._custom_dve(
                        RECIPROCAL_APPROX_FAST, out=rt[:], in0=z_ps[:],
                        s0=cst["s0"], s1=cst["s1"], imm2=cst["imm2"],
                    )
                else:
                    lt = lp.tile([128, k], F32, tag="l")
                    nc.scalar.activation(lt[:], z_ps[:], mybir.ActivationFunctionType.Ln)
                    nc.scalar.activation(rt[:], lt[:], mybir.ActivationFunctionType.Exp,
                                         scale=-1.0)
                j1 = jp.tile([128, 1], BF16, tag="j1")
                nc.vector._custom_dve(
                    DICE_RANGE_REDUCE, out=j1[:].broadcast_to((128, k)),
                    in0=et[:], in1=rt[:],
                    s0=0.0, s1=float(k), accum_out=s_acc[:, t:t + 1],
                )
                j2 = jp.tile([128, 1], BF16, tag="j2")
                nc.vector._custom_dve(
                    DICE_RANGE_REDUCE, out=j2[:].broadcast_to((128, k)),
                    in0=et[:], in1=rt[:],
                    s0=lo_t[:, t:t + 1], s1=hi_t[:, t:t + 1],
                    accum_out=d_acc[:, t:t + 1],
                )
            nc.sync.dma_start(out=s_out[:], in_=s_acc[:])
            nc.sync.dma_start(out=d_out[:], in_=d_acc[:])
    nc.finalize()
    return nc


def host_w1():
    w1 = np.zeros((128, 128), dtype=BF)
    for g in range(G):
        w1[g * 32:(g + 1) * 32, g * 32:(g + 1) * 32] = BF(1.0)
    return w1


def host_prep(preds_b, targets_b, gpix=GPIX, k=2048):
    """Sort pixels by target within each group; build x [128, gpix] plus
    per-tile class-range bounds lo/hi [128, nt]."""
    nt = gpix // k
    xc = preds_b.reshape(C, G, gpix)
    tg = targets_b.reshape(G, gpix)
    x = np.empty((G * C, gpix), dtype=np.float32)
    lo = np.zeros((G * C, nt), dtype=np.float32)
    hi = np.zeros((G * C, nt), dtype=np.float32)
    for g in range(G):
        perm = np.argsort(tg[g], kind="stable")
        x[g * C:(g + 1) * C, :] = xc[:, g, :][:, perm]
        counts = np.bincount(tg[g].astype(np.int64), minlength=C)
        ends = np.cumsum(counts)
        starts = ends - counts
        for t in range(nt):
            lo[g * C:(g + 1) * C, t] = np.clip(starts - t * k, 0, k)
            hi[g * C:(g + 1) * C, t] = np.clip(ends - t * k, 0, k)
    return x, lo, hi


def finish_loss(S, D, Ncnt, npix_total):
    """Host-side 32-vector algebra, mirrors the reference exactly."""
    S = S.astype(np.float64)
    D = D.astype(np.float64)
    Ncnt = Ncnt.astype(np.float64)
    TP = EPS * S + (1.0 - EPS) * D
    FP = S - TP
    FN = (EPS * npix_total + (1.0 - EPS) * Ncnt) - TP
    alpha = np.clip(FP / (FP + FN + SMOOTH), 0.2, 0.8)
    beta = 1.0 - alpha
    den = TP + alpha * FP + beta * FN
    dice = TP / (den + SMOOTH)
    loss = np.sum(1.0 - dice) / C
    return np.float32(loss)


_NC_CACHE = {}


def _get_nc():
    if "nc" not in _NC_CACHE:
        _NC_CACHE["nc"] = build_nc()
    return _NC_CACHE["nc"]


def kernel(preds, targets):
    preds = np.asarray(preds, dtype=np.float32)
    targets = np.asarray(targets)
    nc = _get_nc()
    w1 = host_w1()
    in_maps = []
    for b in range(NCORES):
        x, lo, hi = host_prep(preds[b].reshape(C, HW), targets[b].reshape(-1))
        in_maps.append({"x": x, "lo": lo, "hi": hi, "w1": w1})
    res = run_bass_kernel_spmd(nc, in_maps, list(range(NCORES))).results
    S = np.zeros(C, dtype=np.float64)
    D = np.zeros(C, dtype=np.float64)
    for b in range(NCORES):
        so = np.asarray(res[b]["s_out"], dtype=np.float64)  # [128, nt]
        do = np.asarray(res[b]["d_out"], dtype=np.float64)
        S += so.sum(axis=1).reshape(G, C).sum(axis=0)
        D += do.sum(axis=1).reshape(G, C).sum(axis=0)
    Ncnt = np.bincount(targets.reshape(-1).astype(np.int64), minlength=C).astype(np.float64)
    return np.array(finish_loss(S, D, Ncnt, preds.shape[0] * HW), dtype=np.float32)
